# revision 4
# baseline (speedup 1.0000x reference)
"""Sliding-window causal GQA attention (RoPE) on 8 TRN2 NeuronCores.

Problem: B=2 packed seqs x S=2048, HQ=32 q heads, HK=8 kv heads, D=128,
WINDOW=1024, causal. GQA group size 4.

Sharding: core c owns kv head c and its 4 query heads (zero collectives).

Structure (ScalarE exp is the binding engine at ~92us of pure work/core):
  - q/k arrive pre-transposed from the host ([d, token] layout, q
    head-interleaved to [d, (qi, g, j)]) together with rotate-half copies
    and head-tiled cos/sin, packed per 512-token quarter: qa = [q | cos4]
    (sync HWDGE queue), qb = [qrot | sin4] + kpack = [k | krot] (GpSimd
    SWDGE queue). No on-device transposes; RoPE is six DVE multiplies per
    quarter, pipelined one quarter ahead of the consuming q blocks.
  - mm1 head-batched: one N=512 matmul per (b, qi, chunk) with the kv
    chunk as stationary weights (216 matmuls/core instead of 864).
  - exp batched 3 chunks per ScalarE activation (N=1536, PSUM tiles of 3
    banks, double-buffered) to amortize ACT's ~352-cycle per-instruction
    overhead.
  - triangular masks as pre-tiled [128, 512] bf16 multiplies (DVE),
    emitted right after the exp batch that produces their input.
  - mm2 in the transposed orientation (pt chunk as stationary weights,
    rhs = [v | 1] so the softmax denominator rides along): 4 heads x nch
    chunks per (b, qi) accumulated into two [128, 2, 129] PSUM tiles,
    one contiguous accumulation group per head (interleaving any other
    matmul inside an open PSUM accumulation group corrupts it), masked
    chunks last.
  - software pipelining: the first two mm1+exp batches of block n+1 are
    emitted before block n's mm2 so ScalarE never idles at block
    boundaries (their PSUM WAR dependencies clear exactly one exp earlier).
  - normalize per head on DVE (tensor_scalar by the reciprocal of the
    denominator column), store per block on the GpSimd queue.
"""

import json
import os
import sys

import numpy as np

sys.path.insert(0, "/opt/trn_rl_repo")

import ml_dtypes  # noqa: E402

import concourse.bass as bass  # noqa: E402
import concourse.tile as tile  # noqa: E402
from concourse import mybir  # noqa: E402
from concourse.bass_utils import run_bass_kernel_spmd  # noqa: E402


# ---------------------------------------------------------------------------
# BIR legalization: this environment's walrus build encodes at most ONE sync
# wait (and one update) per instruction.  Tile attaches several.  Hoist the
# extras onto standalone EventSemaphore nops (same engine, just before the
# owning instruction) — identical semantics, raw-bass style.
# ---------------------------------------------------------------------------
def _legalize_bir(bir_json):
    d = json.loads(bir_json)
    for fn in d["functions"]:
        for blk in fn["blocks"]:
            new = []
            for inst in blk["instructions"]:
                si = inst.get("sync_info")
                if si:
                    waits = si.get("on_wait") or []
                    if len(waits) > 1:
                        for j, w in enumerate(waits[:-1]):
                            new.append({
                                "debug": inst.get("debug", 0),
                                "engine": inst["engine"],
                                "ins": [],
                                "outs": [],
                                "name": f"{inst['name']}_hw{j}",
                                "opcode": "EventSemaphore",
                                "sync_info": {"on_update": [], "on_wait": [w]},
                            })
                        si["on_wait"] = [waits[-1]]
                new.append(inst)
            blk["instructions"] = new
    return json.dumps(d).encode()


def _install_legalizer():
    import concourse.bass_utils as _bu
    import concourse.bass2jax as _b2j

    if getattr(_bu, "_single_wait_legalizer", None):
        return
    _orig = _bu.compile_bir_kernel

    def _patched(bir_json, tmpdir, neff_name="file.neff"):
        return _orig(_legalize_bir(bir_json), tmpdir, neff_name=neff_name)

    _bu.compile_bir_kernel = _patched
    _b2j.compile_bir_kernel = _patched
    _bu._single_wait_legalizer = True


_install_legalizer()

BF16 = ml_dtypes.bfloat16

# Problem config (hardcoded per spec)
B, S = 2, 2048
HQ, HK, D = 32, 8, 128
G = HQ // HK  # 4
WINDOW = 1024
THETA = 10000.0
NTOK = B * S  # 4096
NCORES = 8
HALF = D // 2  # 64

NQB = S // 128          # 16 query blocks of 128 per sequence
NKC = S // 128          # 16 kv chunks of 128 per sequence
MAXCH = WINDOW // 128 + 1  # 9: max kv chunks touched by one q block
SCALE = 1.0 / float(np.sqrt(D))
GD = G * D              # 512
EXPB = 3                # chunks per exp batch (3 PSUM banks per st tile)

_CACHED_NC = None


def _build_nc():
    """Build the per-core Bass graph (identical on all 8 cores)."""
    fp32 = mybir.dt.float32
    bf16 = mybir.dt.bfloat16
    nc = bass.Bass()

    # packed transposed inputs, quarter-major:
    #   qa: per (b, qq): [q-quarter | cos4-quarter]   (2 x 2048 cols)
    #   qb: per (b, qq): [qrot-quarter | sin4-quarter]
    #   kc: per (b, qq): [k-quarter | krot-quarter]   (2 x 512 cols)
    qa_ext = nc.declare_dram_parameter("qa", [D, NTOK * G * 2], bf16,
                                       isOutput=False)
    qb_ext = nc.declare_dram_parameter("qb", [D, NTOK * G * 2], bf16,
                                       isOutput=False)
    kc_ext = nc.declare_dram_parameter("kc", [D, NTOK * 2], bf16,
                                       isOutput=False)
    # v pre-arranged to [kv-in-chunk, chunk, d|1] so the load is contiguous
    v_ext = nc.declare_dram_parameter("v", [128, (NTOK // 128) * (D + 1)], bf16,
                                      isOutput=False)
    ctri_ext = nc.declare_dram_parameter("ctri4", [128, GD], bf16, isOutput=False)
    wtri_ext = nc.declare_dram_parameter("wtri4", [128, GD], bf16, isOutput=False)
    out_ext = nc.declare_dram_parameter("out", [NTOK, GD], fp32, isOutput=True)

    with tile.TileContext(nc) as tc:
        from contextlib import ExitStack

        with ExitStack() as ctx:
            const = ctx.enter_context(tc.tile_pool(name="const", bufs=1))
            ropet = ctx.enter_context(tc.tile_pool(name="ropet", bufs=2))
            pt_pool = ctx.enter_context(tc.tile_pool(name="pt", bufs=7))
            osb_pool = ctx.enter_context(tc.tile_pool(name="osb", bufs=3))
            rec_pool = ctx.enter_context(tc.tile_pool(name="rec", bufs=4))
            qa_pool = ctx.enter_context(tc.tile_pool(name="qa", bufs=3))
            qb_pool = ctx.enter_context(tc.tile_pool(name="qb", bufs=3))
            kc_pool = ctx.enter_context(tc.tile_pool(name="kc", bufs=3))
            st_pool = ctx.enter_context(tc.tile_pool(name="st", bufs=2, space="PSUM"))
            po_pool = ctx.enter_context(tc.tile_pool(name="po", bufs=1, space="PSUM"))

            # ---- persistent SBUF tensors ----
            # qT4[b]: [d=128, (qi, g, j)] head-interleaved transposed queries
            qT4 = [const.tile([128, NQB, G, 128], bf16, name=f"qT4b{b}",
                              tag=f"qT4b{b}") for b in range(B)]
            # kT[b]: [d=128, (chunk, j)]
            kT = [const.tile([128, NKC, 128], bf16, name=f"kTb{b}", tag=f"kTb{b}")
                  for b in range(B)]
            vsb = const.tile([128, NTOK // 128, D + 1], bf16)  # [kv_j, chunk, d|1]
            ctri4 = const.tile([128, GD], bf16)
            wtri4 = const.tile([128, GD], bf16)

            packs = {}

            def loads(b, qq, first=False):
                q0 = (b * 4 + qq) * 4096
                k0 = (b * 4 + qq) * 1024
                qa = qa_pool.tile([128, 2, 4, G, 128], bf16, name="qa",
                                  tag="qa")
                qb = qb_pool.tile([128, 2, 4, G, 128], bf16, name="qb",
                                  tag="qb")
                kc = kc_pool.tile([128, 2, 4, 128], bf16, name="kc", tag="kc")
                packs[(b, qq)] = (qa, qb, kc)
                # the second quarter rides the otherwise-idle scalar HWDGE
                # queue so it doesn't serialize behind the first
                qeng = nc.scalar if first else nc.sync
                beng = nc.scalar if first else nc.gpsimd
                qeng.dma_start(
                    qa, qa_ext[:, q0:q0 + 4096].rearrange(
                        "p (k c g j) -> p k c g j", k=2, g=G, j=128))
                beng.dma_start(
                    qb, qb_ext[:, q0:q0 + 4096].rearrange(
                        "p (k c g j) -> p k c g j", k=2, g=G, j=128))
                nc.gpsimd.dma_start(
                    kc, kc_ext[:, k0:k0 + 1024].rearrange(
                        "p (k c j) -> p k c j", k=2, j=128))

            # ---- RoPE one 512-token quarter at a time (pure DVE) ----
            # roped = x * cos_dup + xr * sin_signed; xr is the pre-rotated
            # copy loaded from DRAM, consumed (and clobbered) in place.
            # Emitted as 3 op-groups interleaved with the current quarter's
            # q blocks so the DVE FIFO never stalls behind a 4us rope wall.
            def rope_chunks(b, qq):
                sl = slice(qq * 4, (qq + 1) * 4)
                qa, qb, kc = packs[(b, qq)]
                qx, cs4 = qa[:, 0], qa[:, 1]
                qr, sn4 = qb[:, 0], qb[:, 1]
                cs, sn = cs4[:, :, 0], sn4[:, :, 0]
                tq = [None]

                def c1():
                    tk = ropet.tile([128, 4, 128], bf16, name="tk", tag="tk")
                    nc.vector.tensor_mul(tk, kc[:, 0], cs)
                    nc.vector.tensor_mul(kc[:, 1], kc[:, 1], sn)
                    nc.vector.tensor_add(kT[b][:, sl], tk, kc[:, 1])

                def c2():
                    tq[0] = ropet.tile([128, 4, G, 128], bf16, name="tq",
                                       tag="tq")
                    nc.vector.tensor_mul(tq[0], qx, cs4)

                def c3():
                    nc.vector.tensor_mul(qr, qr, sn4)
                    nc.vector.tensor_add(qT4[b][:, sl], tq[0], qr)

                return [c1, c2, c3]

            # masks first (tiny, on sync) so the PE warmup can start at once
            nc.sync.dma_start(ctri4, ctri_ext[:, :])
            nc.sync.dma_start(wtri4, wtri_ext[:, :])
            # first two quarters spread over all three DMA queues; the big
            # q/qrot packs ride the two fast HWDGE queues (sync + scalar —
            # ScalarE has no exp work yet), the slower GpSimd SWDGE queue
            # only carries the small k packs and the early v chunks
            qa0 = qa_pool.tile([128, 2, 4, G, 128], bf16, name="qa0", tag="qa")
            qb0 = qb_pool.tile([128, 2, 4, G, 128], bf16, name="qb0", tag="qb")
            kc0 = kc_pool.tile([128, 2, 4, 128], bf16, name="kc0", tag="kc")
            packs[(0, 0)] = (qa0, qb0, kc0)
            nc.gpsimd.dma_start(
                kc0, kc_ext[:, 0:1024].rearrange("p (k c j) -> p k c j",
                                                 k=2, j=128))
            nc.gpsimd.dma_start(
                vsb[:, 0:4],
                v_ext[:, 0:4 * (D + 1)].rearrange("p (c d) -> p c d", d=D + 1))
            for ki in range(2):
                h0 = ki * 2048
                h1 = ki * 2048 + 1024
                nc.sync.dma_start(
                    qa0[:, ki, 0:2], qa_ext[:, h0:h0 + 1024].rearrange(
                        "p (c g j) -> p c g j", g=G, j=128))
                nc.scalar.dma_start(
                    qb0[:, ki, 0:2], qb_ext[:, h0:h0 + 1024].rearrange(
                        "p (c g j) -> p c g j", g=G, j=128))
            for ki in range(2):
                h1 = ki * 2048 + 1024
                nc.scalar.dma_start(
                    qa0[:, ki, 2:4], qa_ext[:, h1:h1 + 1024].rearrange(
                        "p (c g j) -> p c g j", g=G, j=128))
                nc.sync.dma_start(
                    qb0[:, ki, 2:4], qb_ext[:, h1:h1 + 1024].rearrange(
                        "p (c g j) -> p c g j", g=G, j=128))
            # quarter 1: qa on sync, qb split scalar/gpsimd, k on gpsimd
            qa1 = qa_pool.tile([128, 2, 4, G, 128], bf16, name="qa1", tag="qa")
            qb1 = qb_pool.tile([128, 2, 4, G, 128], bf16, name="qb1", tag="qb")
            kc1 = kc_pool.tile([128, 2, 4, 128], bf16, name="kc1", tag="kc")
            packs[(0, 1)] = (qa1, qb1, kc1)
            nc.gpsimd.dma_start(
                kc1, kc_ext[:, 1024:2048].rearrange("p (k c j) -> p k c j",
                                                    k=2, j=128))
            nc.sync.dma_start(
                qa1, qa_ext[:, 4096:8192].rearrange(
                    "p (k c g j) -> p k c g j", k=2, g=G, j=128))
            for ki in range(2):
                h0 = 4096 + ki * 2048
                h1 = 4096 + ki * 2048 + 1024
                nc.scalar.dma_start(
                    qb1[:, ki, 0:2], qb_ext[:, h0:h0 + 1024].rearrange(
                        "p (c g j) -> p c g j", g=G, j=128))
                nc.gpsimd.dma_start(
                    qb1[:, ki, 2:4], qb_ext[:, h1:h1 + 1024].rearrange(
                        "p (c g j) -> p c g j", g=G, j=128))
            nc.scalar.dma_start(
                vsb[:, 4:NKC],
                v_ext[:, 4 * (D + 1):NKC * (D + 1)].rearrange(
                    "p (c d) -> p c d", d=D + 1))
            # rope quarter 0 per half so blocks 0-1 go live first
            for hf in range(2):
                c2 = slice(hf * 2, hf * 2 + 2)
                tk = ropet.tile([128, 2, 128], bf16, name="tkh", tag="tkh")
                nc.vector.tensor_mul(tk, kc0[:, 0, c2], qa0[:, 1, c2, 0])
                nc.vector.tensor_mul(kc0[:, 1, c2], kc0[:, 1, c2],
                                     qb0[:, 1, c2, 0])
                nc.vector.tensor_add(kT[0][:, c2], tk, kc0[:, 1, c2])
                tq = ropet.tile([128, 2, G, 128], bf16, name="tqh", tag="tqh")
                nc.vector.tensor_mul(tq, qa0[:, 0, c2], qa0[:, 1, c2])
                nc.vector.tensor_mul(qb0[:, 0, c2], qb0[:, 0, c2],
                                     qb0[:, 1, c2])
                nc.vector.tensor_add(qT4[0][:, c2], tq, qb0[:, 0, c2])

            # PE warmup: keep the HAM clock gate busy during the prologue so
            # the main loop starts at 2.4 GHz (dummy matmuls, discarded).
            warm = st_pool.tile([128, EXPB, GD], fp32, tag="st")
            for i in range(8):
                nc.tensor.matmul(warm[:, i % EXPB], ctri4[:, 0:128], ctri4,
                                 start=True, stop=True)

            # ---- main attention loop, software-pipelined ----
            def mm1_batch(P, bi):
                b, qi, bcs = P["b"], P["qi"], P["batches"][bi]
                st = st_pool.tile([128, EXPB, GD], fp32, tag="st")
                for sj, c in enumerate(bcs):
                    nc.tensor.matmul(
                        st[:, sj],
                        kT[b][:, c],
                        qT4[b][:, qi],
                        start=True,
                        stop=True,
                    )
                pt = pt_pool.tile([128, EXPB, GD], bf16, tag="pt")
                bs = len(bcs)
                nc.scalar.activation(
                    pt[:, 0:bs],
                    st[:, 0:bs],
                    mybir.ActivationFunctionType.Exp,
                    scale=SCALE,
                )
                P["pts"].append(pt)
                # masks as soon as their exp batch exists (keeps the DVE
                # dependency off the mm2 critical path)
                if bi == 0 and P["edge_c"] is not None:
                    nc.vector.tensor_mul(pt[:, 0], pt[:, 0], wtri4)
                lb, ls = divmod(P["nch"] - 1, EXPB)
                if bi == lb:
                    nc.vector.tensor_mul(pt[:, ls], pt[:, ls], ctri4)

            def stage1_open(b, qi):
                c0 = max(0, qi - (MAXCH - 1))
                cs_list = list(range(c0, qi + 1))
                batches = [cs_list[i:i + EXPB] for i in range(0, len(cs_list),
                                                              EXPB)]
                edge_c = c0 if qi >= MAXCH - 1 else None
                mids = [c for c in cs_list if c != qi and c != edge_c]
                tail = ([edge_c] if edge_c is not None else []) + [qi]
                P = dict(b=b, qi=qi, c0=c0, nch=len(cs_list), pts=[],
                         batches=batches, edge_c=edge_c, mids=mids, tail=tail)
                for bi in range(min(2, len(batches))):
                    mm1_batch(P, bi)
                return P

            def stage1_rest(P):
                for bi in range(2, len(P["batches"])):
                    mm1_batch(P, bi)

            def pslice(P, c, g):
                bi, sj = divmod(c - P["c0"], EXPB)
                return P["pts"][bi][:, sj, g * 128:(g + 1) * 128]

            def stage3(P):
                # mm2, one contiguous accumulation group per head; masked
                # chunks last within each group
                po = [po_pool.tile([128, 2, D + 1], fp32, name=f"po{h}",
                                   tag=f"po{h}")
                      for h in range(2)]
                for g in range(G):
                    for idx, c in enumerate(P["mids"] + P["tail"]):
                        nc.tensor.matmul(
                            po[g // 2][:, g % 2],
                            pslice(P, c, g),
                            vsb[:, P["b"] * NKC + c],
                            start=(idx == 0),
                            stop=(idx == P["nch"] - 1),
                        )
                # normalize: reciprocal of the denominator columns, then a
                # per-head scalar multiply into the SBUF out tile
                osb = osb_pool.tile([128, GD], fp32, tag="osb")
                for h in range(2):
                    rec = rec_pool.tile([128, 2], fp32, name=f"rec{h}",
                                        tag=f"rec{h}")
                    nc.vector.reciprocal(rec, po[h][:, :, D:D + 1])
                    for gg in range(2):
                        g = h * 2 + gg
                        nc.vector.tensor_scalar_mul(
                            osb[:, g * 128:(g + 1) * 128],
                            po[h][:, gg, 0:D],
                            rec[:, gg:gg + 1],
                        )
                r0 = P["b"] * S + P["qi"] * 128
                nc.gpsimd.dma_start(out_ext[r0:r0 + 128, :], osb)

            # pipeline: loads two quarters ahead, rope one quarter ahead of
            # the q blocks that consume them (rope op-groups interleaved
            # between this quarter's q blocks)
            steps = [(b, qq) for b in range(B) for qq in range(4)]
            allqi = [(b, qi) for b in range(B) for qi in range(NQB)]
            P = stage1_open(0, 0)
            stage1_rest(P)
            n = 0
            for i, (b, qq) in enumerate(steps):
                if i + 2 < len(steps):
                    loads(*steps[i + 2])
                if (b, qq) == (0, 2):
                    nc.sync.dma_start(
                        vsb[:, NKC:2 * NKC],
                        v_ext[:, NKC * (D + 1):].rearrange("p (c d) -> p c d",
                                                           d=D + 1))
                chunks = rope_chunks(*steps[i + 1]) if i + 1 < len(steps) else []
                for qi in range(qq * 4, qq * 4 + 4):
                    # next quarter's rope op-group must precede the stage1
                    # that consumes it at the quarter boundary
                    if chunks:
                        chunks.pop(0)()
                    Pn = (stage1_open(*allqi[n + 1])
                          if n + 1 < len(allqi) else None)
                    stage3(P)
                    if Pn is not None:
                        stage1_rest(Pn)
                    P = Pn
                    n += 1

    return nc


def _get_nc():
    global _CACHED_NC
    if _CACHED_NC is None:
        _CACHED_NC = _build_nc()
    return _CACHED_NC


def _host_tables(positions):
    """Rotary cos/sin caches in transposed-dup layout + triangular masks."""
    pos = positions.astype(np.float32)  # [NTOK]
    invf = (1.0 / (THETA ** (np.arange(HALF, dtype=np.float32) / HALF)))  # [64]
    ang = pos[None, :] * invf[:, None]  # [64, NTOK]
    c = np.cos(ang)
    s = np.sin(ang)
    cosd = np.concatenate([c, c], axis=0).astype(BF16)          # [128, NTOK]
    sind = np.concatenate([-s, s], axis=0).astype(BF16)         # [128, NTOK]
    # tile over the interleaved head dim -> [128, (b, qi, g, j)]
    cosd = np.ascontiguousarray(
        np.broadcast_to(cosd.reshape(128, B * NQB, 1, 128),
                        (128, B * NQB, G, 128)).reshape(128, NTOK * G))
    sind = np.ascontiguousarray(
        np.broadcast_to(sind.reshape(128, B * NQB, 1, 128),
                        (128, B * NQB, G, 128)).reshape(128, NTOK * G))
    p = np.arange(128)[:, None]
    f = np.arange(128)[None, :]
    ctri = (p <= f).astype(BF16)   # causal diagonal chunk: keep j<=i
    wtri = (f < p).astype(BF16)    # window edge chunk: keep i-j<WINDOW
    ctri4 = np.tile(ctri, (1, G))  # [128, 512]: per-head copies
    wtri4 = np.tile(wtri, (1, G))
    return cosd, sind, ctri4, wtri4


def _rot(xt):
    """Rotate-half along the (leading) d axis of a [d, token] array."""
    return np.concatenate([xt[HALF:], xt[:HALF]], axis=0)


def _run(inputs, trace=False):
    query = inputs["query"]
    key = inputs["key"]
    value = inputs["value"]
    positions = inputs["positions"]

    cosd, sind, ctri4, wtri4 = _host_tables(positions)
    qf = query.astype(BF16)
    kf = key.astype(BF16)
    vf = value.astype(BF16)
    ones = np.ones((NTOK, 1), dtype=BF16)

    def quarters(x):
        # [128, NTOK*G] -> [128, 8 quarters, 2048]
        return x.reshape(128, 8, 2048)

    in_maps = []
    for c in range(NCORES):
        # head-interleave to rows (b, qi, g, j), then transpose to [d, *]
        qc = qf[:, c * GD:(c + 1) * GD]
        q2 = (qc.reshape(B, NQB, 128, G, D)
              .transpose(0, 1, 3, 2, 4).reshape(NTOK * G, D))
        qt = np.ascontiguousarray(q2.T)
        kt = np.ascontiguousarray(kf[:, c * D:(c + 1) * D].T)
        qtr, ktr = _rot(qt), _rot(kt)
        # pack per quarter: qa = [q | cos4], qb = [qrot | sin4],
        # kc = [k | krot]
        qa_p = np.stack([quarters(qt), quarters(cosd)], axis=2)
        qb_p = np.stack([quarters(qtr), quarters(sind)], axis=2)
        kc_p = np.stack([kt.reshape(128, 8, 512), ktr.reshape(128, 8, 512)],
                        axis=2)
        in_maps.append({
            "qa": np.ascontiguousarray(qa_p.reshape(128, NTOK * G * 2)),
            "qb": np.ascontiguousarray(qb_p.reshape(128, NTOK * G * 2)),
            "kc": np.ascontiguousarray(kc_p.reshape(128, NTOK * 2)),
            "v": np.ascontiguousarray(
                np.concatenate([vf[:, c * D:(c + 1) * D], ones], axis=1)
                .reshape(NTOK // 128, 128, D + 1).transpose(1, 0, 2)
                .reshape(128, (NTOK // 128) * (D + 1))
            ),
            "ctri4": ctri4,
            "wtri4": wtri4,
        })

    nc = _get_nc()
    res = run_bass_kernel_spmd(nc, in_maps, core_ids=list(range(NCORES)),
                               trace=trace)
    out = np.concatenate([res.results[c]["out"] for c in range(NCORES)], axis=1)
    return out.astype(np.float32), res


def kernel(query, key, value, positions):
    out, _ = _run({"query": query, "key": key, "value": value,
                   "positions": positions},
                  trace=bool(os.environ.get("KERNEL_TRACE")))
    return out


# revision 5
# speedup vs baseline: 1.0317x; 1.0317x over previous
"""Sliding-window causal GQA attention (RoPE) on 8 TRN2 NeuronCores.

Problem: B=2 packed seqs x S=2048, HQ=32 q heads, HK=8 kv heads, D=128,
WINDOW=1024, causal. GQA group size 4.

Sharding: core c owns kv head c and its 4 query heads (zero collectives).

Structure (ScalarE exp is the binding engine at ~92us of pure work/core):
  - q/k arrive pre-transposed from the host ([d, token] layout, q
    head-interleaved to [d, (qi, g, j)]) together with rotate-half copies
    and head-tiled cos/sin, packed per 512-token quarter: qa = [q | cos4]
    (sync HWDGE queue), qb = [qrot | sin4] + kpack = [k | krot] (GpSimd
    SWDGE queue). No on-device transposes; RoPE is six DVE multiplies per
    quarter, pipelined one quarter ahead of the consuming q blocks.
  - mm1 head-batched: one N=512 matmul per (b, qi, chunk) with the kv
    chunk as stationary weights (216 matmuls/core instead of 864).
  - exp batched 3 chunks per ScalarE activation (N=1536, PSUM tiles of 3
    banks, double-buffered) to amortize ACT's ~352-cycle per-instruction
    overhead.
  - triangular masks as pre-tiled [128, 512] bf16 multiplies (DVE),
    emitted right after the exp batch that produces their input.
  - mm2 in the transposed orientation (pt chunk as stationary weights,
    rhs = [v | 1] so the softmax denominator rides along): 4 heads x nch
    chunks per (b, qi) accumulated into two [128, 2, 129] PSUM tiles,
    one contiguous accumulation group per head (interleaving any other
    matmul inside an open PSUM accumulation group corrupts it), masked
    chunks last.
  - software pipelining: the first two mm1+exp batches of block n+1 are
    emitted before block n's mm2 so ScalarE never idles at block
    boundaries (their PSUM WAR dependencies clear exactly one exp earlier).
  - normalize per head on DVE (tensor_scalar by the reciprocal of the
    denominator column), store per block on the GpSimd queue.
"""

import json
import os
import sys

import numpy as np

sys.path.insert(0, "/opt/trn_rl_repo")

import ml_dtypes  # noqa: E402

import concourse.bass as bass  # noqa: E402
import concourse.tile as tile  # noqa: E402
from concourse import mybir  # noqa: E402
from concourse.bass_utils import run_bass_kernel_spmd  # noqa: E402


# ---------------------------------------------------------------------------
# BIR legalization: this environment's walrus build encodes at most ONE sync
# wait (and one update) per instruction.  Tile attaches several.  Hoist the
# extras onto standalone EventSemaphore nops (same engine, just before the
# owning instruction) — identical semantics, raw-bass style.
# ---------------------------------------------------------------------------
def _legalize_bir(bir_json):
    d = json.loads(bir_json)
    for fn in d["functions"]:
        for blk in fn["blocks"]:
            new = []
            for inst in blk["instructions"]:
                si = inst.get("sync_info")
                if si:
                    waits = si.get("on_wait") or []
                    if len(waits) > 1:
                        for j, w in enumerate(waits[:-1]):
                            new.append({
                                "debug": inst.get("debug", 0),
                                "engine": inst["engine"],
                                "ins": [],
                                "outs": [],
                                "name": f"{inst['name']}_hw{j}",
                                "opcode": "EventSemaphore",
                                "sync_info": {"on_update": [], "on_wait": [w]},
                            })
                        si["on_wait"] = [waits[-1]]
                new.append(inst)
            blk["instructions"] = new
    return json.dumps(d).encode()


def _install_legalizer():
    import concourse.bass_utils as _bu
    import concourse.bass2jax as _b2j

    if getattr(_bu, "_single_wait_legalizer", None):
        return
    _orig = _bu.compile_bir_kernel

    def _patched(bir_json, tmpdir, neff_name="file.neff"):
        return _orig(_legalize_bir(bir_json), tmpdir, neff_name=neff_name)

    _bu.compile_bir_kernel = _patched
    _b2j.compile_bir_kernel = _patched
    _bu._single_wait_legalizer = True


_install_legalizer()

BF16 = ml_dtypes.bfloat16

# Problem config (hardcoded per spec)
B, S = 2, 2048
HQ, HK, D = 32, 8, 128
G = HQ // HK  # 4
WINDOW = 1024
THETA = 10000.0
NTOK = B * S  # 4096
NCORES = 8
HALF = D // 2  # 64

NQB = S // 128          # 16 query blocks of 128 per sequence
NKC = S // 128          # 16 kv chunks of 128 per sequence
MAXCH = WINDOW // 128 + 1  # 9: max kv chunks touched by one q block
SCALE = 1.0 / float(np.sqrt(D))
GD = G * D              # 512
EXPB = 3                # chunks per exp batch (3 PSUM banks per st tile)

_CACHED_NC = None


def _build_nc():
    """Build the per-core Bass graph (identical on all 8 cores)."""
    fp32 = mybir.dt.float32
    bf16 = mybir.dt.bfloat16
    nc = bass.Bass()

    # packed transposed inputs, quarter-major:
    #   qa: per (b, qq): [q-quarter | cos4-quarter]   (2 x 2048 cols)
    #   qb: per (b, qq): [qrot-quarter | sin4-quarter]
    #   kc: per (b, qq): [k-quarter | krot-quarter]   (2 x 512 cols)
    qa_ext = nc.declare_dram_parameter("qa", [D, NTOK * G * 2], bf16,
                                       isOutput=False)
    qb_ext = nc.declare_dram_parameter("qb", [D, NTOK * G * 2], bf16,
                                       isOutput=False)
    kc_ext = nc.declare_dram_parameter("kc", [D, NTOK * 2], bf16,
                                       isOutput=False)
    # v pre-arranged to [kv-in-chunk, chunk, d|1] so the load is contiguous
    v_ext = nc.declare_dram_parameter("v", [128, (NTOK // 128) * (D + 1)], bf16,
                                      isOutput=False)
    ctri_ext = nc.declare_dram_parameter("ctri4", [128, GD], bf16, isOutput=False)
    wtri_ext = nc.declare_dram_parameter("wtri4", [128, GD], bf16, isOutput=False)
    out_ext = nc.declare_dram_parameter("out", [NTOK, GD], fp32, isOutput=True)

    with tile.TileContext(nc) as tc:
        from contextlib import ExitStack

        with ExitStack() as ctx:
            const = ctx.enter_context(tc.tile_pool(name="const", bufs=1))
            ropet = ctx.enter_context(tc.tile_pool(name="ropet", bufs=3))
            pt_pool = ctx.enter_context(tc.tile_pool(name="pt", bufs=8))
            osb_pool = ctx.enter_context(tc.tile_pool(name="osb", bufs=4))
            rec_pool = ctx.enter_context(tc.tile_pool(name="rec", bufs=8))
            qa_pool = ctx.enter_context(tc.tile_pool(name="qa", bufs=3))
            qb_pool = ctx.enter_context(tc.tile_pool(name="qb", bufs=3))
            kc_pool = ctx.enter_context(tc.tile_pool(name="kc", bufs=3))
            st_pool = ctx.enter_context(tc.tile_pool(name="st", bufs=2, space="PSUM"))
            po_pool = ctx.enter_context(tc.tile_pool(name="po", bufs=1, space="PSUM"))

            # ---- persistent SBUF tensors ----
            # qT4[b]: [d=128, (qi, g, j)] head-interleaved transposed queries
            qT4 = [const.tile([128, NQB, G, 128], bf16, name=f"qT4b{b}",
                              tag=f"qT4b{b}") for b in range(B)]
            # kT[b]: [d=128, (chunk, j)]
            kT = [const.tile([128, NKC, 128], bf16, name=f"kTb{b}", tag=f"kTb{b}")
                  for b in range(B)]
            vsb = const.tile([128, NTOK // 128, D + 1], bf16)  # [kv_j, chunk, d|1]
            ctri4 = const.tile([128, GD], bf16)
            wtri4 = const.tile([128, GD], bf16)

            packs = {}

            def loads(b, qq, first=False):
                q0 = (b * 4 + qq) * 4096
                k0 = (b * 4 + qq) * 1024
                qa = qa_pool.tile([128, 2, 4, G, 128], bf16, name="qa",
                                  tag="qa")
                qb = qb_pool.tile([128, 2, 4, G, 128], bf16, name="qb",
                                  tag="qb")
                kc = kc_pool.tile([128, 2, 4, 128], bf16, name="kc", tag="kc")
                packs[(b, qq)] = (qa, qb, kc)
                # the second quarter rides the otherwise-idle scalar HWDGE
                # queue so it doesn't serialize behind the first
                qeng = nc.scalar if first else nc.sync
                beng = nc.scalar if first else nc.gpsimd
                qeng.dma_start(
                    qa, qa_ext[:, q0:q0 + 4096].rearrange(
                        "p (k c g j) -> p k c g j", k=2, g=G, j=128))
                beng.dma_start(
                    qb, qb_ext[:, q0:q0 + 4096].rearrange(
                        "p (k c g j) -> p k c g j", k=2, g=G, j=128))
                nc.gpsimd.dma_start(
                    kc, kc_ext[:, k0:k0 + 1024].rearrange(
                        "p (k c j) -> p k c j", k=2, j=128))

            # ---- RoPE one 512-token quarter at a time (pure DVE) ----
            # roped = x * cos_dup + xr * sin_signed; xr is the pre-rotated
            # copy loaded from DRAM, consumed (and clobbered) in place.
            # Emitted as 3 op-groups interleaved with the current quarter's
            # q blocks so the DVE FIFO never stalls behind a 4us rope wall.
            def rope_chunks(b, qq):
                sl = slice(qq * 4, (qq + 1) * 4)
                qa, qb, kc = packs[(b, qq)]
                qx, cs4 = qa[:, 0], qa[:, 1]
                qr, sn4 = qb[:, 0], qb[:, 1]
                cs, sn = cs4[:, :, 0], sn4[:, :, 0]
                tq = [None]

                def c1():
                    tk = ropet.tile([128, 4, 128], bf16, name="tk", tag="tk")
                    nc.vector.tensor_mul(tk, kc[:, 0], cs)
                    nc.vector.tensor_mul(kc[:, 1], kc[:, 1], sn)
                    nc.vector.tensor_add(kT[b][:, sl], tk, kc[:, 1])

                def c2():
                    tq[0] = ropet.tile([128, 4, G, 128], bf16, name="tq",
                                       tag="tq")
                    nc.vector.tensor_mul(tq[0], qx, cs4)

                def c3():
                    nc.vector.tensor_mul(qr, qr, sn4)
                    nc.vector.tensor_add(qT4[b][:, sl], tq[0], qr)

                return [c1, c2, c3]

            # masks first (tiny, on sync) so the PE warmup can start at once
            nc.sync.dma_start(ctri4, ctri_ext[:, :])
            nc.sync.dma_start(wtri4, wtri_ext[:, :])
            # first two quarters spread over all three DMA queues; the big
            # q/qrot packs ride the two fast HWDGE queues (sync + scalar —
            # ScalarE has no exp work yet), the slower GpSimd SWDGE queue
            # only carries the small k packs and the early v chunks
            qa0 = qa_pool.tile([128, 2, 4, G, 128], bf16, name="qa0", tag="qa")
            qb0 = qb_pool.tile([128, 2, 4, G, 128], bf16, name="qb0", tag="qb")
            kc0 = kc_pool.tile([128, 2, 4, 128], bf16, name="kc0", tag="kc")
            packs[(0, 0)] = (qa0, qb0, kc0)
            nc.gpsimd.dma_start(
                kc0, kc_ext[:, 0:1024].rearrange("p (k c j) -> p k c j",
                                                 k=2, j=128))
            nc.gpsimd.dma_start(
                vsb[:, 0:4],
                v_ext[:, 0:4 * (D + 1)].rearrange("p (c d) -> p c d", d=D + 1))
            for ki in range(2):
                h0 = ki * 2048
                h1 = ki * 2048 + 1024
                nc.sync.dma_start(
                    qa0[:, ki, 0:2], qa_ext[:, h0:h0 + 1024].rearrange(
                        "p (c g j) -> p c g j", g=G, j=128))
                nc.scalar.dma_start(
                    qb0[:, ki, 0:2], qb_ext[:, h0:h0 + 1024].rearrange(
                        "p (c g j) -> p c g j", g=G, j=128))
            for ki in range(2):
                h1 = ki * 2048 + 1024
                nc.scalar.dma_start(
                    qa0[:, ki, 2:4], qa_ext[:, h1:h1 + 1024].rearrange(
                        "p (c g j) -> p c g j", g=G, j=128))
                nc.sync.dma_start(
                    qb0[:, ki, 2:4], qb_ext[:, h1:h1 + 1024].rearrange(
                        "p (c g j) -> p c g j", g=G, j=128))
            # quarter 1: qa on sync, qb split scalar/gpsimd, k on gpsimd
            qa1 = qa_pool.tile([128, 2, 4, G, 128], bf16, name="qa1", tag="qa")
            qb1 = qb_pool.tile([128, 2, 4, G, 128], bf16, name="qb1", tag="qb")
            kc1 = kc_pool.tile([128, 2, 4, 128], bf16, name="kc1", tag="kc")
            packs[(0, 1)] = (qa1, qb1, kc1)
            nc.gpsimd.dma_start(
                kc1, kc_ext[:, 1024:2048].rearrange("p (k c j) -> p k c j",
                                                    k=2, j=128))
            nc.sync.dma_start(
                qa1, qa_ext[:, 4096:8192].rearrange(
                    "p (k c g j) -> p k c g j", k=2, g=G, j=128))
            for ki in range(2):
                h0 = 4096 + ki * 2048
                h1 = 4096 + ki * 2048 + 1024
                nc.scalar.dma_start(
                    qb1[:, ki, 0:2], qb_ext[:, h0:h0 + 1024].rearrange(
                        "p (c g j) -> p c g j", g=G, j=128))
                nc.gpsimd.dma_start(
                    qb1[:, ki, 2:4], qb_ext[:, h1:h1 + 1024].rearrange(
                        "p (c g j) -> p c g j", g=G, j=128))
            nc.scalar.dma_start(
                vsb[:, 4:NKC],
                v_ext[:, 4 * (D + 1):NKC * (D + 1)].rearrange(
                    "p (c d) -> p c d", d=D + 1))
            # rope quarter 0 per half so blocks 0-1 go live first
            for hf in range(2):
                c2 = slice(hf * 2, hf * 2 + 2)
                tk = ropet.tile([128, 2, 128], bf16, name="tkh", tag="tkh")
                nc.vector.tensor_mul(tk, kc0[:, 0, c2], qa0[:, 1, c2, 0])
                nc.vector.tensor_mul(kc0[:, 1, c2], kc0[:, 1, c2],
                                     qb0[:, 1, c2, 0])
                nc.vector.tensor_add(kT[0][:, c2], tk, kc0[:, 1, c2])
                tq = ropet.tile([128, 2, G, 128], bf16, name="tqh", tag="tqh")
                nc.vector.tensor_mul(tq, qa0[:, 0, c2], qa0[:, 1, c2])
                nc.vector.tensor_mul(qb0[:, 0, c2], qb0[:, 0, c2],
                                     qb0[:, 1, c2])
                nc.vector.tensor_add(qT4[0][:, c2], tq, qb0[:, 0, c2])

            # PE warmup: keep the HAM clock gate busy during the prologue so
            # the main loop starts at 2.4 GHz (dummy matmuls, discarded).
            warm = st_pool.tile([128, EXPB, GD], fp32, tag="st")
            for i in range(8):
                nc.tensor.matmul(warm[:, i % EXPB], ctri4[:, 0:128], ctri4,
                                 start=True, stop=True)

            # ---- main attention loop, software-pipelined ----
            def mm1_batch(P, bi):
                b, qi, bcs = P["b"], P["qi"], P["batches"][bi]
                st = st_pool.tile([128, EXPB, GD], fp32, tag="st")
                for sj, c in enumerate(bcs):
                    nc.tensor.matmul(
                        st[:, sj],
                        kT[b][:, c],
                        qT4[b][:, qi],
                        start=True,
                        stop=True,
                    )
                pt = pt_pool.tile([128, EXPB, GD], bf16, tag="pt")
                bs = len(bcs)
                nc.scalar.activation(
                    pt[:, 0:bs],
                    st[:, 0:bs],
                    mybir.ActivationFunctionType.Exp,
                    scale=SCALE,
                )
                P["pts"].append(pt)
                # masks as soon as their exp batch exists (keeps the DVE
                # dependency off the mm2 critical path)
                if bi == 0 and P["edge_c"] is not None:
                    nc.vector.tensor_mul(pt[:, 0], pt[:, 0], wtri4)
                lb, ls = divmod(P["nch"] - 1, EXPB)
                if bi == lb:
                    nc.vector.tensor_mul(pt[:, ls], pt[:, ls], ctri4)

            def stage1_open(b, qi):
                c0 = max(0, qi - (MAXCH - 1))
                cs_list = list(range(c0, qi + 1))
                batches = [cs_list[i:i + EXPB] for i in range(0, len(cs_list),
                                                              EXPB)]
                edge_c = c0 if qi >= MAXCH - 1 else None
                mids = [c for c in cs_list if c != qi and c != edge_c]
                tail = ([edge_c] if edge_c is not None else []) + [qi]
                P = dict(b=b, qi=qi, c0=c0, nch=len(cs_list), pts=[],
                         batches=batches, edge_c=edge_c, mids=mids, tail=tail)
                for bi in range(min(2, len(batches))):
                    mm1_batch(P, bi)
                return P

            def stage1_rest(P):
                for bi in range(2, len(P["batches"])):
                    mm1_batch(P, bi)

            def pslice(P, c, g):
                bi, sj = divmod(c - P["c0"], EXPB)
                return P["pts"][bi][:, sj, g * 128:(g + 1) * 128]

            def stage3(P):
                # mm2, one contiguous accumulation group per head; masked
                # chunks last within each group
                po = [po_pool.tile([128, 2, D + 1], fp32, name=f"po{h}",
                                   tag=f"po{h}")
                      for h in range(2)]
                for g in range(G):
                    for idx, c in enumerate(P["mids"] + P["tail"]):
                        nc.tensor.matmul(
                            po[g // 2][:, g % 2],
                            pslice(P, c, g),
                            vsb[:, P["b"] * NKC + c],
                            start=(idx == 0),
                            stop=(idx == P["nch"] - 1),
                        )
                # normalize: reciprocal of the denominator columns, then a
                # per-head scalar multiply into the SBUF out tile
                osb = osb_pool.tile([128, GD], fp32, tag="osb")
                for h in range(2):
                    rec = rec_pool.tile([128, 2], fp32, name=f"rec{h}",
                                        tag=f"rec{h}")
                    nc.vector.reciprocal(rec, po[h][:, :, D:D + 1])
                    for gg in range(2):
                        g = h * 2 + gg
                        nc.vector.tensor_scalar_mul(
                            osb[:, g * 128:(g + 1) * 128],
                            po[h][:, gg, 0:D],
                            rec[:, gg:gg + 1],
                        )
                r0 = P["b"] * S + P["qi"] * 128
                nc.gpsimd.dma_start(out_ext[r0:r0 + 128, :], osb)

            # pipeline: loads two quarters ahead, rope one quarter ahead of
            # the q blocks that consume them (rope op-groups interleaved
            # between this quarter's q blocks)
            steps = [(b, qq) for b in range(B) for qq in range(4)]
            allqi = [(b, qi) for b in range(B) for qi in range(NQB)]
            P = stage1_open(0, 0)
            stage1_rest(P)
            n = 0
            for i, (b, qq) in enumerate(steps):
                if i + 2 < len(steps):
                    loads(*steps[i + 2])
                if (b, qq) == (0, 2):
                    nc.sync.dma_start(
                        vsb[:, NKC:2 * NKC],
                        v_ext[:, NKC * (D + 1):].rearrange("p (c d) -> p c d",
                                                           d=D + 1))
                chunks = rope_chunks(*steps[i + 1]) if i + 1 < len(steps) else []
                for qi in range(qq * 4, qq * 4 + 4):
                    # next quarter's rope op-group must precede the stage1
                    # that consumes it at the quarter boundary
                    if chunks:
                        chunks.pop(0)()
                    Pn = (stage1_open(*allqi[n + 1])
                          if n + 1 < len(allqi) else None)
                    stage3(P)
                    if Pn is not None:
                        stage1_rest(Pn)
                    P = Pn
                    n += 1

    return nc


def _get_nc():
    global _CACHED_NC
    if _CACHED_NC is None:
        _CACHED_NC = _build_nc()
    return _CACHED_NC


def _host_tables(positions):
    """Rotary cos/sin caches in transposed-dup layout + triangular masks."""
    pos = positions.astype(np.float32)  # [NTOK]
    invf = (1.0 / (THETA ** (np.arange(HALF, dtype=np.float32) / HALF)))  # [64]
    ang = pos[None, :] * invf[:, None]  # [64, NTOK]
    c = np.cos(ang)
    s = np.sin(ang)
    cosd = np.concatenate([c, c], axis=0).astype(BF16)          # [128, NTOK]
    sind = np.concatenate([-s, s], axis=0).astype(BF16)         # [128, NTOK]
    # tile over the interleaved head dim -> [128, (b, qi, g, j)]
    cosd = np.ascontiguousarray(
        np.broadcast_to(cosd.reshape(128, B * NQB, 1, 128),
                        (128, B * NQB, G, 128)).reshape(128, NTOK * G))
    sind = np.ascontiguousarray(
        np.broadcast_to(sind.reshape(128, B * NQB, 1, 128),
                        (128, B * NQB, G, 128)).reshape(128, NTOK * G))
    p = np.arange(128)[:, None]
    f = np.arange(128)[None, :]
    ctri = (p <= f).astype(BF16)   # causal diagonal chunk: keep j<=i
    wtri = (f < p).astype(BF16)    # window edge chunk: keep i-j<WINDOW
    ctri4 = np.tile(ctri, (1, G))  # [128, 512]: per-head copies
    wtri4 = np.tile(wtri, (1, G))
    return cosd, sind, ctri4, wtri4


def _rot(xt):
    """Rotate-half along the (leading) d axis of a [d, token] array."""
    return np.concatenate([xt[HALF:], xt[:HALF]], axis=0)


def _run(inputs, trace=False):
    query = inputs["query"]
    key = inputs["key"]
    value = inputs["value"]
    positions = inputs["positions"]

    cosd, sind, ctri4, wtri4 = _host_tables(positions)
    qf = query.astype(BF16)
    kf = key.astype(BF16)
    vf = value.astype(BF16)
    ones = np.ones((NTOK, 1), dtype=BF16)

    def quarters(x):
        # [128, NTOK*G] -> [128, 8 quarters, 2048]
        return x.reshape(128, 8, 2048)

    in_maps = []
    for c in range(NCORES):
        # head-interleave to rows (b, qi, g, j), then transpose to [d, *]
        qc = qf[:, c * GD:(c + 1) * GD]
        q2 = (qc.reshape(B, NQB, 128, G, D)
              .transpose(0, 1, 3, 2, 4).reshape(NTOK * G, D))
        qt = np.ascontiguousarray(q2.T)
        kt = np.ascontiguousarray(kf[:, c * D:(c + 1) * D].T)
        qtr, ktr = _rot(qt), _rot(kt)
        # pack per quarter: qa = [q | cos4], qb = [qrot | sin4],
        # kc = [k | krot]
        qa_p = np.stack([quarters(qt), quarters(cosd)], axis=2)
        qb_p = np.stack([quarters(qtr), quarters(sind)], axis=2)
        kc_p = np.stack([kt.reshape(128, 8, 512), ktr.reshape(128, 8, 512)],
                        axis=2)
        in_maps.append({
            "qa": np.ascontiguousarray(qa_p.reshape(128, NTOK * G * 2)),
            "qb": np.ascontiguousarray(qb_p.reshape(128, NTOK * G * 2)),
            "kc": np.ascontiguousarray(kc_p.reshape(128, NTOK * 2)),
            "v": np.ascontiguousarray(
                np.concatenate([vf[:, c * D:(c + 1) * D], ones], axis=1)
                .reshape(NTOK // 128, 128, D + 1).transpose(1, 0, 2)
                .reshape(128, (NTOK // 128) * (D + 1))
            ),
            "ctri4": ctri4,
            "wtri4": wtri4,
        })

    nc = _get_nc()
    res = run_bass_kernel_spmd(nc, in_maps, core_ids=list(range(NCORES)),
                               trace=trace)
    out = np.concatenate([res.results[c]["out"] for c in range(NCORES)], axis=1)
    return out.astype(np.float32), res


def kernel(query, key, value, positions):
    out, _ = _run({"query": query, "key": key, "value": value,
                   "positions": positions},
                  trace=bool(os.environ.get("KERNEL_TRACE")))
    return out


# revision 6
# speedup vs baseline: 1.0458x; 1.0137x over previous
"""Sliding-window causal GQA attention (RoPE) on 8 TRN2 NeuronCores.

Problem: B=2 packed seqs x S=2048, HQ=32 q heads, HK=8 kv heads, D=128,
WINDOW=1024, causal. GQA group size 4.

Sharding: core c owns kv head c and its 4 query heads (zero collectives).

Structure (ScalarE exp is the binding engine at ~92us of pure work/core):
  - q/k arrive pre-transposed from the host ([d, token] layout, q
    head-interleaved to [d, (qi, g, j)]) together with rotate-half copies
    and head-tiled cos/sin, packed per 512-token quarter: qa = [q | cos4]
    (sync HWDGE queue), qb = [qrot | sin4] + kpack = [k | krot] (GpSimd
    SWDGE queue). No on-device transposes; RoPE is six DVE multiplies per
    quarter, pipelined one quarter ahead of the consuming q blocks.
  - mm1 head-batched: one N=512 matmul per (b, qi, chunk) with the kv
    chunk as stationary weights (216 matmuls/core instead of 864).
  - exp batched 3 chunks per ScalarE activation (N=1536, PSUM tiles of 3
    banks, double-buffered) to amortize ACT's ~352-cycle per-instruction
    overhead.
  - triangular masks as pre-tiled [128, 512] bf16 multiplies (DVE),
    emitted right after the exp batch that produces their input.
  - mm2 in the transposed orientation (pt chunk as stationary weights,
    rhs = [v | 1] so the softmax denominator rides along): 4 heads x nch
    chunks per (b, qi) accumulated into two [128, 2, 129] PSUM tiles,
    one contiguous accumulation group per head (interleaving any other
    matmul inside an open PSUM accumulation group corrupts it), masked
    chunks last.
  - software pipelining: the first two mm1+exp batches of block n+1 are
    emitted before block n's mm2 so ScalarE never idles at block
    boundaries (their PSUM WAR dependencies clear exactly one exp earlier).
  - normalize per head on DVE (tensor_scalar by the reciprocal of the
    denominator column), store per block on the sync queue (the GpSimd
    SWDGE queue is slower and stays dedicated to the qb packs).
"""

import json
import os
import sys

import numpy as np

sys.path.insert(0, "/opt/trn_rl_repo")

import ml_dtypes  # noqa: E402

import concourse.bass as bass  # noqa: E402
import concourse.tile as tile  # noqa: E402
from concourse import mybir  # noqa: E402
from concourse.bass_utils import run_bass_kernel_spmd  # noqa: E402


# ---------------------------------------------------------------------------
# BIR legalization: this environment's walrus build encodes at most ONE sync
# wait (and one update) per instruction.  Tile attaches several.  Hoist the
# extras onto standalone EventSemaphore nops (same engine, just before the
# owning instruction) — identical semantics, raw-bass style.
# ---------------------------------------------------------------------------
def _legalize_bir(bir_json):
    d = json.loads(bir_json)
    for fn in d["functions"]:
        for blk in fn["blocks"]:
            new = []
            for inst in blk["instructions"]:
                si = inst.get("sync_info")
                if si:
                    waits = si.get("on_wait") or []
                    if len(waits) > 1:
                        for j, w in enumerate(waits[:-1]):
                            new.append({
                                "debug": inst.get("debug", 0),
                                "engine": inst["engine"],
                                "ins": [],
                                "outs": [],
                                "name": f"{inst['name']}_hw{j}",
                                "opcode": "EventSemaphore",
                                "sync_info": {"on_update": [], "on_wait": [w]},
                            })
                        si["on_wait"] = [waits[-1]]
                new.append(inst)
            blk["instructions"] = new
    return json.dumps(d).encode()


def _install_legalizer():
    import concourse.bass_utils as _bu
    import concourse.bass2jax as _b2j

    if getattr(_bu, "_single_wait_legalizer", None):
        return
    _orig = _bu.compile_bir_kernel

    def _patched(bir_json, tmpdir, neff_name="file.neff"):
        return _orig(_legalize_bir(bir_json), tmpdir, neff_name=neff_name)

    _bu.compile_bir_kernel = _patched
    _b2j.compile_bir_kernel = _patched
    _bu._single_wait_legalizer = True


_install_legalizer()

BF16 = ml_dtypes.bfloat16

# Problem config (hardcoded per spec)
B, S = 2, 2048
HQ, HK, D = 32, 8, 128
G = HQ // HK  # 4
WINDOW = 1024
THETA = 10000.0
NTOK = B * S  # 4096
NCORES = 8
HALF = D // 2  # 64

NQB = S // 128          # 16 query blocks of 128 per sequence
NKC = S // 128          # 16 kv chunks of 128 per sequence
MAXCH = WINDOW // 128 + 1  # 9: max kv chunks touched by one q block
SCALE = 1.0 / float(np.sqrt(D))
GD = G * D              # 512
EXPB = 3                # chunks per exp batch (3 PSUM banks per st tile)

_CACHED_NC = None


def _build_nc():
    """Build the per-core Bass graph (identical on all 8 cores)."""
    fp32 = mybir.dt.float32
    bf16 = mybir.dt.bfloat16
    nc = bass.Bass()

    # packed transposed inputs, quarter-major:
    #   qa: per (b, qq): [q-quarter | cos4-quarter]   (2 x 2048 cols)
    #   qb: per (b, qq): [qrot-quarter | sin4-quarter]
    #   kc: per (b, qq): [k-quarter | krot-quarter]   (2 x 512 cols)
    qa_ext = nc.declare_dram_parameter("qa", [D, NTOK * G * 2], bf16,
                                       isOutput=False)
    qb_ext = nc.declare_dram_parameter("qb", [D, NTOK * G * 2], bf16,
                                       isOutput=False)
    kc_ext = nc.declare_dram_parameter("kc", [D, NTOK * 2], bf16,
                                       isOutput=False)
    # v pre-arranged to [kv-in-chunk, chunk, d|1] so the load is contiguous
    v_ext = nc.declare_dram_parameter("v", [128, (NTOK // 128) * (D + 1)], bf16,
                                      isOutput=False)
    ctri_ext = nc.declare_dram_parameter("ctri4", [128, GD], bf16, isOutput=False)
    wtri_ext = nc.declare_dram_parameter("wtri4", [128, GD], bf16, isOutput=False)
    out_ext = nc.declare_dram_parameter("out", [NTOK, GD], fp32, isOutput=True)

    with tile.TileContext(nc) as tc:
        from contextlib import ExitStack

        with ExitStack() as ctx:
            const = ctx.enter_context(tc.tile_pool(name="const", bufs=1))
            ropet = ctx.enter_context(tc.tile_pool(name="ropet", bufs=3))
            pt_pool = ctx.enter_context(tc.tile_pool(name="pt", bufs=8))
            osb_pool = ctx.enter_context(tc.tile_pool(name="osb", bufs=4))
            rec_pool = ctx.enter_context(tc.tile_pool(name="rec", bufs=8))
            qa_pool = ctx.enter_context(tc.tile_pool(name="qa", bufs=3))
            qb_pool = ctx.enter_context(tc.tile_pool(name="qb", bufs=3))
            kc_pool = ctx.enter_context(tc.tile_pool(name="kc", bufs=3))
            st_pool = ctx.enter_context(tc.tile_pool(name="st", bufs=2, space="PSUM"))
            po_pool = ctx.enter_context(tc.tile_pool(name="po", bufs=1, space="PSUM"))

            # ---- persistent SBUF tensors ----
            # qT4[b]: [d=128, (qi, g, j)] head-interleaved transposed queries
            qT4 = [const.tile([128, NQB, G, 128], bf16, name=f"qT4b{b}",
                              tag=f"qT4b{b}") for b in range(B)]
            # kT[b]: [d=128, (chunk, j)]
            kT = [const.tile([128, NKC, 128], bf16, name=f"kTb{b}", tag=f"kTb{b}")
                  for b in range(B)]
            vsb = const.tile([128, NTOK // 128, D + 1], bf16)  # [kv_j, chunk, d|1]
            ctri4 = const.tile([128, GD], bf16)
            wtri4 = const.tile([128, GD], bf16)

            packs = {}

            def loads(b, qq, first=False):
                q0 = (b * 4 + qq) * 4096
                k0 = (b * 4 + qq) * 1024
                qa = qa_pool.tile([128, 2, 4, G, 128], bf16, name="qa",
                                  tag="qa")
                qb = qb_pool.tile([128, 2, 4, G, 128], bf16, name="qb",
                                  tag="qb")
                kc = kc_pool.tile([128, 2, 4, 128], bf16, name="kc", tag="kc")
                packs[(b, qq)] = (qa, qb, kc)
                # the second quarter rides the otherwise-idle scalar HWDGE
                # queue so it doesn't serialize behind the first
                qeng = nc.scalar if first else nc.sync
                beng = nc.scalar if first else nc.gpsimd
                qeng.dma_start(
                    qa, qa_ext[:, q0:q0 + 4096].rearrange(
                        "p (k c g j) -> p k c g j", k=2, g=G, j=128))
                beng.dma_start(
                    qb, qb_ext[:, q0:q0 + 4096].rearrange(
                        "p (k c g j) -> p k c g j", k=2, g=G, j=128))
                nc.sync.dma_start(
                    kc, kc_ext[:, k0:k0 + 1024].rearrange(
                        "p (k c j) -> p k c j", k=2, j=128))

            # ---- RoPE one 512-token quarter at a time (pure DVE) ----
            # roped = x * cos_dup + xr * sin_signed; xr is the pre-rotated
            # copy loaded from DRAM, consumed (and clobbered) in place.
            # Emitted as 3 op-groups interleaved with the current quarter's
            # q blocks so the DVE FIFO never stalls behind a 4us rope wall.
            def rope_chunks(b, qq):
                sl = slice(qq * 4, (qq + 1) * 4)
                qa, qb, kc = packs[(b, qq)]
                qx, cs4 = qa[:, 0], qa[:, 1]
                qr, sn4 = qb[:, 0], qb[:, 1]
                cs, sn = cs4[:, :, 0], sn4[:, :, 0]
                tq = [None]

                def c1():
                    tk = ropet.tile([128, 4, 128], bf16, name="tk", tag="tk")
                    nc.vector.tensor_mul(tk, kc[:, 0], cs)
                    nc.vector.tensor_mul(kc[:, 1], kc[:, 1], sn)
                    nc.vector.tensor_add(kT[b][:, sl], tk, kc[:, 1])

                def c2():
                    tq[0] = ropet.tile([128, 4, G, 128], bf16, name="tq",
                                       tag="tq")
                    nc.vector.tensor_mul(tq[0], qx, cs4)

                def c3():
                    nc.vector.tensor_mul(qr, qr, sn4)
                    nc.vector.tensor_add(qT4[b][:, sl], tq[0], qr)

                return [c1, c2, c3]

            # masks first (tiny, on sync) so the PE warmup can start at once
            nc.sync.dma_start(ctri4, ctri_ext[:, :])
            nc.sync.dma_start(wtri4, wtri_ext[:, :])
            # first two quarters spread over all three DMA queues; the big
            # q/qrot packs ride the two fast HWDGE queues (sync + scalar —
            # ScalarE has no exp work yet), the slower GpSimd SWDGE queue
            # only carries the small k packs and the early v chunks
            qa0 = qa_pool.tile([128, 2, 4, G, 128], bf16, name="qa0", tag="qa")
            qb0 = qb_pool.tile([128, 2, 4, G, 128], bf16, name="qb0", tag="qb")
            kc0 = kc_pool.tile([128, 2, 4, 128], bf16, name="kc0", tag="kc")
            packs[(0, 0)] = (qa0, qb0, kc0)
            nc.gpsimd.dma_start(
                kc0, kc_ext[:, 0:1024].rearrange("p (k c j) -> p k c j",
                                                 k=2, j=128))
            nc.gpsimd.dma_start(
                vsb[:, 0:4],
                v_ext[:, 0:4 * (D + 1)].rearrange("p (c d) -> p c d", d=D + 1))
            for ki in range(2):
                h0 = ki * 2048
                h1 = ki * 2048 + 1024
                nc.sync.dma_start(
                    qa0[:, ki, 0:2], qa_ext[:, h0:h0 + 1024].rearrange(
                        "p (c g j) -> p c g j", g=G, j=128))
                nc.scalar.dma_start(
                    qb0[:, ki, 0:2], qb_ext[:, h0:h0 + 1024].rearrange(
                        "p (c g j) -> p c g j", g=G, j=128))
            for ki in range(2):
                h1 = ki * 2048 + 1024
                nc.scalar.dma_start(
                    qa0[:, ki, 2:4], qa_ext[:, h1:h1 + 1024].rearrange(
                        "p (c g j) -> p c g j", g=G, j=128))
                nc.sync.dma_start(
                    qb0[:, ki, 2:4], qb_ext[:, h1:h1 + 1024].rearrange(
                        "p (c g j) -> p c g j", g=G, j=128))
            # quarter 1: qa on sync, qb split scalar/gpsimd, k on gpsimd
            qa1 = qa_pool.tile([128, 2, 4, G, 128], bf16, name="qa1", tag="qa")
            qb1 = qb_pool.tile([128, 2, 4, G, 128], bf16, name="qb1", tag="qb")
            kc1 = kc_pool.tile([128, 2, 4, 128], bf16, name="kc1", tag="kc")
            packs[(0, 1)] = (qa1, qb1, kc1)
            nc.gpsimd.dma_start(
                kc1, kc_ext[:, 1024:2048].rearrange("p (k c j) -> p k c j",
                                                    k=2, j=128))
            nc.sync.dma_start(
                qa1, qa_ext[:, 4096:8192].rearrange(
                    "p (k c g j) -> p k c g j", k=2, g=G, j=128))
            for ki in range(2):
                h0 = 4096 + ki * 2048
                h1 = 4096 + ki * 2048 + 1024
                nc.scalar.dma_start(
                    qb1[:, ki, 0:2], qb_ext[:, h0:h0 + 1024].rearrange(
                        "p (c g j) -> p c g j", g=G, j=128))
                nc.gpsimd.dma_start(
                    qb1[:, ki, 2:4], qb_ext[:, h1:h1 + 1024].rearrange(
                        "p (c g j) -> p c g j", g=G, j=128))
            nc.scalar.dma_start(
                vsb[:, 4:NKC],
                v_ext[:, 4 * (D + 1):NKC * (D + 1)].rearrange(
                    "p (c d) -> p c d", d=D + 1))
            # rope quarter 0 per half so blocks 0-1 go live first
            for hf in range(2):
                c2 = slice(hf * 2, hf * 2 + 2)
                tk = ropet.tile([128, 2, 128], bf16, name="tkh", tag="tkh")
                nc.vector.tensor_mul(tk, kc0[:, 0, c2], qa0[:, 1, c2, 0])
                nc.vector.tensor_mul(kc0[:, 1, c2], kc0[:, 1, c2],
                                     qb0[:, 1, c2, 0])
                nc.vector.tensor_add(kT[0][:, c2], tk, kc0[:, 1, c2])
                tq = ropet.tile([128, 2, G, 128], bf16, name="tqh", tag="tqh")
                nc.vector.tensor_mul(tq, qa0[:, 0, c2], qa0[:, 1, c2])
                nc.vector.tensor_mul(qb0[:, 0, c2], qb0[:, 0, c2],
                                     qb0[:, 1, c2])
                nc.vector.tensor_add(qT4[0][:, c2], tq, qb0[:, 0, c2])

            # PE warmup: keep the HAM clock gate busy during the prologue so
            # the main loop starts at 2.4 GHz (dummy matmuls, discarded).
            warm = st_pool.tile([128, EXPB, GD], fp32, tag="st")
            for i in range(8):
                nc.tensor.matmul(warm[:, i % EXPB], ctri4[:, 0:128], ctri4,
                                 start=True, stop=True)

            # ---- main attention loop, software-pipelined ----
            def mm1_batch(P, bi):
                b, qi, bcs = P["b"], P["qi"], P["batches"][bi]
                st = st_pool.tile([128, EXPB, GD], fp32, tag="st")
                for sj, c in enumerate(bcs):
                    nc.tensor.matmul(
                        st[:, sj],
                        kT[b][:, c],
                        qT4[b][:, qi],
                        start=True,
                        stop=True,
                    )
                pt = pt_pool.tile([128, EXPB, GD], bf16, tag="pt")
                bs = len(bcs)
                nc.scalar.activation(
                    pt[:, 0:bs],
                    st[:, 0:bs],
                    mybir.ActivationFunctionType.Exp,
                    scale=SCALE,
                )
                P["pts"].append(pt)
                # masks as soon as their exp batch exists (keeps the DVE
                # dependency off the mm2 critical path)
                if bi == 0 and P["edge_c"] is not None:
                    nc.vector.tensor_mul(pt[:, 0], pt[:, 0], wtri4)
                lb, ls = divmod(P["nch"] - 1, EXPB)
                if bi == lb:
                    nc.vector.tensor_mul(pt[:, ls], pt[:, ls], ctri4)

            def stage1_open(b, qi):
                c0 = max(0, qi - (MAXCH - 1))
                cs_list = list(range(c0, qi + 1))
                batches = [cs_list[i:i + EXPB] for i in range(0, len(cs_list),
                                                              EXPB)]
                edge_c = c0 if qi >= MAXCH - 1 else None
                mids = [c for c in cs_list if c != qi and c != edge_c]
                tail = ([edge_c] if edge_c is not None else []) + [qi]
                P = dict(b=b, qi=qi, c0=c0, nch=len(cs_list), pts=[],
                         batches=batches, edge_c=edge_c, mids=mids, tail=tail)
                for bi in range(min(2, len(batches))):
                    mm1_batch(P, bi)
                return P

            def stage1_rest(P):
                for bi in range(2, len(P["batches"])):
                    mm1_batch(P, bi)

            def pslice(P, c, g):
                bi, sj = divmod(c - P["c0"], EXPB)
                return P["pts"][bi][:, sj, g * 128:(g + 1) * 128]

            def stage3(P):
                # mm2, one contiguous accumulation group per head; masked
                # chunks last within each group
                po = [po_pool.tile([128, 2, D + 1], fp32, name=f"po{h}",
                                   tag=f"po{h}")
                      for h in range(2)]
                for g in range(G):
                    for idx, c in enumerate(P["mids"] + P["tail"]):
                        nc.tensor.matmul(
                            po[g // 2][:, g % 2],
                            pslice(P, c, g),
                            vsb[:, P["b"] * NKC + c],
                            start=(idx == 0),
                            stop=(idx == P["nch"] - 1),
                        )
                # normalize: reciprocal of the denominator columns, then a
                # per-head scalar multiply into the SBUF out tile
                osb = osb_pool.tile([128, GD], fp32, tag="osb")
                for h in range(2):
                    rec = rec_pool.tile([128, 2], fp32, name=f"rec{h}",
                                        tag=f"rec{h}")
                    nc.vector.reciprocal(rec, po[h][:, :, D:D + 1])
                    for gg in range(2):
                        g = h * 2 + gg
                        nc.vector.tensor_scalar_mul(
                            osb[:, g * 128:(g + 1) * 128],
                            po[h][:, gg, 0:D],
                            rec[:, gg:gg + 1],
                        )
                r0 = P["b"] * S + P["qi"] * 128
                nc.sync.dma_start(out_ext[r0:r0 + 128, :], osb)

            # pipeline: loads two quarters ahead, rope one quarter ahead of
            # the q blocks that consume them (rope op-groups interleaved
            # between this quarter's q blocks)
            steps = [(b, qq) for b in range(B) for qq in range(4)]
            allqi = [(b, qi) for b in range(B) for qi in range(NQB)]
            P = stage1_open(0, 0)
            stage1_rest(P)
            n = 0
            for i, (b, qq) in enumerate(steps):
                if i + 2 < len(steps):
                    loads(*steps[i + 2])
                if (b, qq) == (0, 2):
                    nc.sync.dma_start(
                        vsb[:, NKC:2 * NKC],
                        v_ext[:, NKC * (D + 1):].rearrange("p (c d) -> p c d",
                                                           d=D + 1))
                chunks = rope_chunks(*steps[i + 1]) if i + 1 < len(steps) else []
                for qi in range(qq * 4, qq * 4 + 4):
                    # next quarter's rope op-group must precede the stage1
                    # that consumes it at the quarter boundary
                    if chunks:
                        chunks.pop(0)()
                    Pn = (stage1_open(*allqi[n + 1])
                          if n + 1 < len(allqi) else None)
                    stage3(P)
                    if Pn is not None:
                        stage1_rest(Pn)
                    P = Pn
                    n += 1

    return nc


def _get_nc():
    global _CACHED_NC
    if _CACHED_NC is None:
        _CACHED_NC = _build_nc()
    return _CACHED_NC


def _host_tables(positions):
    """Rotary cos/sin caches in transposed-dup layout + triangular masks."""
    pos = positions.astype(np.float32)  # [NTOK]
    invf = (1.0 / (THETA ** (np.arange(HALF, dtype=np.float32) / HALF)))  # [64]
    ang = pos[None, :] * invf[:, None]  # [64, NTOK]
    c = np.cos(ang)
    s = np.sin(ang)
    cosd = np.concatenate([c, c], axis=0).astype(BF16)          # [128, NTOK]
    sind = np.concatenate([-s, s], axis=0).astype(BF16)         # [128, NTOK]
    # tile over the interleaved head dim -> [128, (b, qi, g, j)]
    cosd = np.ascontiguousarray(
        np.broadcast_to(cosd.reshape(128, B * NQB, 1, 128),
                        (128, B * NQB, G, 128)).reshape(128, NTOK * G))
    sind = np.ascontiguousarray(
        np.broadcast_to(sind.reshape(128, B * NQB, 1, 128),
                        (128, B * NQB, G, 128)).reshape(128, NTOK * G))
    p = np.arange(128)[:, None]
    f = np.arange(128)[None, :]
    ctri = (p <= f).astype(BF16)   # causal diagonal chunk: keep j<=i
    wtri = (f < p).astype(BF16)    # window edge chunk: keep i-j<WINDOW
    ctri4 = np.tile(ctri, (1, G))  # [128, 512]: per-head copies
    wtri4 = np.tile(wtri, (1, G))
    return cosd, sind, ctri4, wtri4


def _rot(xt):
    """Rotate-half along the (leading) d axis of a [d, token] array."""
    return np.concatenate([xt[HALF:], xt[:HALF]], axis=0)


def _run(inputs, trace=False):
    query = inputs["query"]
    key = inputs["key"]
    value = inputs["value"]
    positions = inputs["positions"]

    cosd, sind, ctri4, wtri4 = _host_tables(positions)
    qf = query.astype(BF16)
    kf = key.astype(BF16)
    vf = value.astype(BF16)
    ones = np.ones((NTOK, 1), dtype=BF16)

    def quarters(x):
        # [128, NTOK*G] -> [128, 8 quarters, 2048]
        return x.reshape(128, 8, 2048)

    in_maps = []
    for c in range(NCORES):
        # head-interleave to rows (b, qi, g, j), then transpose to [d, *]
        qc = qf[:, c * GD:(c + 1) * GD]
        q2 = (qc.reshape(B, NQB, 128, G, D)
              .transpose(0, 1, 3, 2, 4).reshape(NTOK * G, D))
        qt = np.ascontiguousarray(q2.T)
        kt = np.ascontiguousarray(kf[:, c * D:(c + 1) * D].T)
        qtr, ktr = _rot(qt), _rot(kt)
        # pack per quarter: qa = [q | cos4], qb = [qrot | sin4],
        # kc = [k | krot]
        qa_p = np.stack([quarters(qt), quarters(cosd)], axis=2)
        qb_p = np.stack([quarters(qtr), quarters(sind)], axis=2)
        kc_p = np.stack([kt.reshape(128, 8, 512), ktr.reshape(128, 8, 512)],
                        axis=2)
        in_maps.append({
            "qa": np.ascontiguousarray(qa_p.reshape(128, NTOK * G * 2)),
            "qb": np.ascontiguousarray(qb_p.reshape(128, NTOK * G * 2)),
            "kc": np.ascontiguousarray(kc_p.reshape(128, NTOK * 2)),
            "v": np.ascontiguousarray(
                np.concatenate([vf[:, c * D:(c + 1) * D], ones], axis=1)
                .reshape(NTOK // 128, 128, D + 1).transpose(1, 0, 2)
                .reshape(128, (NTOK // 128) * (D + 1))
            ),
            "ctri4": ctri4,
            "wtri4": wtri4,
        })

    nc = _get_nc()
    res = run_bass_kernel_spmd(nc, in_maps, core_ids=list(range(NCORES)),
                               trace=trace)
    out = np.concatenate([res.results[c]["out"] for c in range(NCORES)], axis=1)
    return out.astype(np.float32), res


def kernel(query, key, value, positions):
    out, _ = _run({"query": query, "key": key, "value": value,
                   "positions": positions},
                  trace=bool(os.environ.get("KERNEL_TRACE")))
    return out


# revision 7
# speedup vs baseline: 1.0477x; 1.0018x over previous
"""Sliding-window causal GQA attention (RoPE) on 8 TRN2 NeuronCores.

Problem: B=2 packed seqs x S=2048, HQ=32 q heads, HK=8 kv heads, D=128,
WINDOW=1024, causal. GQA group size 4.

Sharding: core c owns kv head c and its 4 query heads (zero collectives).

Structure (ScalarE exp is the binding engine at ~92us of pure work/core):
  - q/k arrive pre-transposed from the host ([d, token] layout, q
    head-interleaved to [d, (qi, g, j)]) together with rotate-half copies
    and head-tiled cos/sin, packed per 512-token quarter: qa = [q | cos4]
    (sync HWDGE queue), qb = [qrot | sin4] + kpack = [k | krot] (GpSimd
    SWDGE queue). No on-device transposes; RoPE is six DVE multiplies per
    quarter, pipelined one quarter ahead of the consuming q blocks.
  - mm1 head-batched: one N=512 matmul per (b, qi, chunk) with the kv
    chunk as stationary weights (216 matmuls/core instead of 864).
  - exp batched 3 chunks per ScalarE activation (N=1536, PSUM tiles of 3
    banks, double-buffered) to amortize ACT's ~352-cycle per-instruction
    overhead.
  - triangular masks as pre-tiled [128, 512] bf16 multiplies (DVE),
    emitted right after the exp batch that produces their input.
  - mm2 in the transposed orientation (pt chunk as stationary weights,
    rhs = [v | 1] so the softmax denominator rides along): 4 heads x nch
    chunks per (b, qi) accumulated into two [128, 2, 129] PSUM tiles,
    one contiguous accumulation group per head (interleaving any other
    matmul inside an open PSUM accumulation group corrupts it), masked
    chunks last.
  - software pipelining: the first two mm1+exp batches of block n+1 are
    emitted before block n's mm2 so ScalarE never idles at block
    boundaries (their PSUM WAR dependencies clear exactly one exp earlier).
  - normalize per head on DVE (tensor_scalar by the reciprocal of the
    denominator column), store per block on the sync queue (the GpSimd
    SWDGE queue is slower and stays dedicated to the qb packs).
"""

import json
import os
import sys

import numpy as np

sys.path.insert(0, "/opt/trn_rl_repo")

import ml_dtypes  # noqa: E402

import concourse.bass as bass  # noqa: E402
import concourse.tile as tile  # noqa: E402
from concourse import mybir  # noqa: E402
from concourse.bass_utils import run_bass_kernel_spmd  # noqa: E402


# ---------------------------------------------------------------------------
# BIR legalization: this environment's walrus build encodes at most ONE sync
# wait (and one update) per instruction.  Tile attaches several.  Hoist the
# extras onto standalone EventSemaphore nops (same engine, just before the
# owning instruction) — identical semantics, raw-bass style.
# ---------------------------------------------------------------------------
def _legalize_bir(bir_json):
    d = json.loads(bir_json)
    for fn in d["functions"]:
        for blk in fn["blocks"]:
            new = []
            for inst in blk["instructions"]:
                si = inst.get("sync_info")
                if si:
                    waits = si.get("on_wait") or []
                    if len(waits) > 1:
                        for j, w in enumerate(waits[:-1]):
                            new.append({
                                "debug": inst.get("debug", 0),
                                "engine": inst["engine"],
                                "ins": [],
                                "outs": [],
                                "name": f"{inst['name']}_hw{j}",
                                "opcode": "EventSemaphore",
                                "sync_info": {"on_update": [], "on_wait": [w]},
                            })
                        si["on_wait"] = [waits[-1]]
                new.append(inst)
            blk["instructions"] = new
    return json.dumps(d).encode()


def _install_legalizer():
    import concourse.bass_utils as _bu
    import concourse.bass2jax as _b2j

    if getattr(_bu, "_single_wait_legalizer", None):
        return
    _orig = _bu.compile_bir_kernel

    def _patched(bir_json, tmpdir, neff_name="file.neff"):
        return _orig(_legalize_bir(bir_json), tmpdir, neff_name=neff_name)

    _bu.compile_bir_kernel = _patched
    _b2j.compile_bir_kernel = _patched
    _bu._single_wait_legalizer = True


_install_legalizer()

BF16 = ml_dtypes.bfloat16

# Problem config (hardcoded per spec)
B, S = 2, 2048
HQ, HK, D = 32, 8, 128
G = HQ // HK  # 4
WINDOW = 1024
THETA = 10000.0
NTOK = B * S  # 4096
NCORES = 8
HALF = D // 2  # 64

NQB = S // 128          # 16 query blocks of 128 per sequence
NKC = S // 128          # 16 kv chunks of 128 per sequence
MAXCH = WINDOW // 128 + 1  # 9: max kv chunks touched by one q block
SCALE = 1.0 / float(np.sqrt(D))
GD = G * D              # 512
EXPB = 3                # chunks per exp batch (3 PSUM banks per st tile)

_CACHED_NC = None


def _build_nc():
    """Build the per-core Bass graph (identical on all 8 cores)."""
    fp32 = mybir.dt.float32
    bf16 = mybir.dt.bfloat16
    nc = bass.Bass()

    # packed transposed inputs, quarter-major:
    #   qa: per (b, qq): [q-quarter | cos4-quarter]   (2 x 2048 cols)
    #   qb: per (b, qq): [qrot-quarter | sin4-quarter]
    #   kc: per (b, qq): [k-quarter | krot-quarter]   (2 x 512 cols)
    qa_ext = nc.declare_dram_parameter("qa", [D, NTOK * G * 2], bf16,
                                       isOutput=False)
    qb_ext = nc.declare_dram_parameter("qb", [D, NTOK * G * 2], bf16,
                                       isOutput=False)
    kc_ext = nc.declare_dram_parameter("kc", [D, NTOK * 2], bf16,
                                       isOutput=False)
    # v pre-arranged to [kv-in-chunk, chunk, d|1] so the load is contiguous
    v_ext = nc.declare_dram_parameter("v", [128, (NTOK // 128) * (D + 1)], bf16,
                                      isOutput=False)
    ctri_ext = nc.declare_dram_parameter("ctri4", [128, GD], bf16, isOutput=False)
    wtri_ext = nc.declare_dram_parameter("wtri4", [128, GD], bf16, isOutput=False)
    out_ext = nc.declare_dram_parameter("out", [NTOK, GD], fp32, isOutput=True)

    with tile.TileContext(nc) as tc:
        from contextlib import ExitStack

        with ExitStack() as ctx:
            const = ctx.enter_context(tc.tile_pool(name="const", bufs=1))
            ropet = ctx.enter_context(tc.tile_pool(name="ropet", bufs=3))
            pt_pool = ctx.enter_context(tc.tile_pool(name="pt", bufs=8))
            osb_pool = ctx.enter_context(tc.tile_pool(name="osb", bufs=4))
            rec_pool = ctx.enter_context(tc.tile_pool(name="rec", bufs=8))
            qa_pool = ctx.enter_context(tc.tile_pool(name="qa", bufs=3))
            qb_pool = ctx.enter_context(tc.tile_pool(name="qb", bufs=3))
            kc_pool = ctx.enter_context(tc.tile_pool(name="kc", bufs=3))
            st_pool = ctx.enter_context(tc.tile_pool(name="st", bufs=2, space="PSUM"))
            po_pool = ctx.enter_context(tc.tile_pool(name="po", bufs=1, space="PSUM"))

            # ---- persistent SBUF tensors ----
            # qT4[b]: [d=128, (qi, g, j)] head-interleaved transposed queries
            qT4 = [const.tile([128, NQB, G, 128], bf16, name=f"qT4b{b}",
                              tag=f"qT4b{b}") for b in range(B)]
            # kT[b]: [d=128, (chunk, j)]
            kT = [const.tile([128, NKC, 128], bf16, name=f"kTb{b}", tag=f"kTb{b}")
                  for b in range(B)]
            vsb = const.tile([128, NTOK // 128, D + 1], bf16)  # [kv_j, chunk, d|1]
            ctri4 = const.tile([128, GD], bf16)
            wtri4 = const.tile([128, GD], bf16)

            packs = {}

            def loads(b, qq, first=False):
                q0 = (b * 4 + qq) * 4096
                k0 = (b * 4 + qq) * 1024
                qa = qa_pool.tile([128, 2, 4, G, 128], bf16, name="qa",
                                  tag="qa")
                qb = qb_pool.tile([128, 2, 4, G, 128], bf16, name="qb",
                                  tag="qb")
                kc = kc_pool.tile([128, 2, 4, 128], bf16, name="kc", tag="kc")
                packs[(b, qq)] = (qa, qb, kc)
                # the second quarter rides the otherwise-idle scalar HWDGE
                # queue so it doesn't serialize behind the first
                qeng = nc.scalar if first else nc.sync
                beng = nc.scalar if first else nc.gpsimd
                qeng.dma_start(
                    qa, qa_ext[:, q0:q0 + 4096].rearrange(
                        "p (k c g j) -> p k c g j", k=2, g=G, j=128))
                beng.dma_start(
                    qb, qb_ext[:, q0:q0 + 4096].rearrange(
                        "p (k c g j) -> p k c g j", k=2, g=G, j=128))
                nc.sync.dma_start(
                    kc, kc_ext[:, k0:k0 + 1024].rearrange(
                        "p (k c j) -> p k c j", k=2, j=128))

            # ---- RoPE one 512-token quarter at a time (pure DVE) ----
            # roped = x * cos_dup + xr * sin_signed; xr is the pre-rotated
            # copy loaded from DRAM, consumed (and clobbered) in place.
            # Emitted as 3 op-groups interleaved with the current quarter's
            # q blocks so the DVE FIFO never stalls behind a 4us rope wall.
            def rope_chunks(b, qq):
                sl = slice(qq * 4, (qq + 1) * 4)
                qa, qb, kc = packs[(b, qq)]
                qx, cs4 = qa[:, 0], qa[:, 1]
                qr, sn4 = qb[:, 0], qb[:, 1]
                cs, sn = cs4[:, :, 0], sn4[:, :, 0]
                tq = [None]

                def c1():
                    tk = ropet.tile([128, 4, 128], bf16, name="tk", tag="tk")
                    nc.vector.tensor_mul(tk, kc[:, 0], cs)
                    nc.vector.tensor_mul(kc[:, 1], kc[:, 1], sn)
                    nc.vector.tensor_add(kT[b][:, sl], tk, kc[:, 1])

                def c2():
                    tq[0] = ropet.tile([128, 4, G, 128], bf16, name="tq",
                                       tag="tq")
                    nc.vector.tensor_mul(tq[0], qx, cs4)

                def c3():
                    nc.vector.tensor_mul(qr, qr, sn4)
                    nc.vector.tensor_add(qT4[b][:, sl], tq[0], qr)

                return [c1, c2, c3]

            # masks first (tiny, on sync) so the PE warmup can start at once
            nc.sync.dma_start(ctri4, ctri_ext[:, :])
            nc.sync.dma_start(wtri4, wtri_ext[:, :])
            # first two quarters spread over all three DMA queues; the big
            # q/qrot packs ride the two fast HWDGE queues (sync + scalar —
            # ScalarE has no exp work yet), the slower GpSimd SWDGE queue
            # only carries the small k packs and the early v chunks
            qa0 = qa_pool.tile([128, 2, 4, G, 128], bf16, name="qa0", tag="qa")
            qb0 = qb_pool.tile([128, 2, 4, G, 128], bf16, name="qb0", tag="qb")
            kc0 = kc_pool.tile([128, 2, 4, 128], bf16, name="kc0", tag="kc")
            packs[(0, 0)] = (qa0, qb0, kc0)
            nc.gpsimd.dma_start(
                kc0, kc_ext[:, 0:1024].rearrange("p (k c j) -> p k c j",
                                                 k=2, j=128))
            nc.gpsimd.dma_start(
                vsb[:, 0:4],
                v_ext[:, 0:4 * (D + 1)].rearrange("p (c d) -> p c d", d=D + 1))
            for ki in range(2):
                h0 = ki * 2048
                h1 = ki * 2048 + 1024
                nc.sync.dma_start(
                    qa0[:, ki, 0:2], qa_ext[:, h0:h0 + 1024].rearrange(
                        "p (c g j) -> p c g j", g=G, j=128))
                nc.scalar.dma_start(
                    qb0[:, ki, 0:2], qb_ext[:, h0:h0 + 1024].rearrange(
                        "p (c g j) -> p c g j", g=G, j=128))
            for ki in range(2):
                h1 = ki * 2048 + 1024
                nc.scalar.dma_start(
                    qa0[:, ki, 2:4], qa_ext[:, h1:h1 + 1024].rearrange(
                        "p (c g j) -> p c g j", g=G, j=128))
                nc.sync.dma_start(
                    qb0[:, ki, 2:4], qb_ext[:, h1:h1 + 1024].rearrange(
                        "p (c g j) -> p c g j", g=G, j=128))
            # quarter 1: qa on sync, qb split scalar/gpsimd, k on gpsimd
            qa1 = qa_pool.tile([128, 2, 4, G, 128], bf16, name="qa1", tag="qa")
            qb1 = qb_pool.tile([128, 2, 4, G, 128], bf16, name="qb1", tag="qb")
            kc1 = kc_pool.tile([128, 2, 4, 128], bf16, name="kc1", tag="kc")
            packs[(0, 1)] = (qa1, qb1, kc1)
            nc.gpsimd.dma_start(
                kc1, kc_ext[:, 1024:2048].rearrange("p (k c j) -> p k c j",
                                                    k=2, j=128))
            for ki in range(2):
                h0 = 4096 + ki * 2048
                h1 = 4096 + ki * 2048 + 1024
                nc.sync.dma_start(
                    qa1[:, ki, 0:2], qa_ext[:, h0:h0 + 1024].rearrange(
                        "p (c g j) -> p c g j", g=G, j=128))
                nc.gpsimd.dma_start(
                    qa1[:, ki, 2:4], qa_ext[:, h1:h1 + 1024].rearrange(
                        "p (c g j) -> p c g j", g=G, j=128))
                nc.scalar.dma_start(
                    qb1[:, ki, 0:2], qb_ext[:, h0:h0 + 1024].rearrange(
                        "p (c g j) -> p c g j", g=G, j=128))
                nc.sync.dma_start(
                    qb1[:, ki, 2:4], qb_ext[:, h1:h1 + 1024].rearrange(
                        "p (c g j) -> p c g j", g=G, j=128))
            nc.scalar.dma_start(
                vsb[:, 4:NKC],
                v_ext[:, 4 * (D + 1):NKC * (D + 1)].rearrange(
                    "p (c d) -> p c d", d=D + 1))
            # rope quarter 0 per half so blocks 0-1 go live first
            for hf in range(2):
                c2 = slice(hf * 2, hf * 2 + 2)
                tk = ropet.tile([128, 2, 128], bf16, name="tkh", tag="tkh")
                nc.vector.tensor_mul(tk, kc0[:, 0, c2], qa0[:, 1, c2, 0])
                nc.vector.tensor_mul(kc0[:, 1, c2], kc0[:, 1, c2],
                                     qb0[:, 1, c2, 0])
                nc.vector.tensor_add(kT[0][:, c2], tk, kc0[:, 1, c2])
                tq = ropet.tile([128, 2, G, 128], bf16, name="tqh", tag="tqh")
                nc.vector.tensor_mul(tq, qa0[:, 0, c2], qa0[:, 1, c2])
                nc.vector.tensor_mul(qb0[:, 0, c2], qb0[:, 0, c2],
                                     qb0[:, 1, c2])
                nc.vector.tensor_add(qT4[0][:, c2], tq, qb0[:, 0, c2])

            # PE warmup: keep the HAM clock gate busy during the prologue so
            # the main loop starts at 2.4 GHz (dummy matmuls, discarded).
            warm = st_pool.tile([128, EXPB, GD], fp32, tag="st")
            for i in range(8):
                nc.tensor.matmul(warm[:, i % EXPB], ctri4[:, 0:128], ctri4,
                                 start=True, stop=True)

            # ---- main attention loop, software-pipelined ----
            def mm1_batch(P, bi):
                b, qi, bcs = P["b"], P["qi"], P["batches"][bi]
                st = st_pool.tile([128, EXPB, GD], fp32, tag="st")
                for sj, c in enumerate(bcs):
                    nc.tensor.matmul(
                        st[:, sj],
                        kT[b][:, c],
                        qT4[b][:, qi],
                        start=True,
                        stop=True,
                    )
                pt = pt_pool.tile([128, EXPB, GD], bf16, tag="pt")
                bs = len(bcs)
                nc.scalar.activation(
                    pt[:, 0:bs],
                    st[:, 0:bs],
                    mybir.ActivationFunctionType.Exp,
                    scale=SCALE,
                )
                P["pts"].append(pt)
                # masks as soon as their exp batch exists (keeps the DVE
                # dependency off the mm2 critical path)
                if bi == 0 and P["edge_c"] is not None:
                    nc.vector.tensor_mul(pt[:, 0], pt[:, 0], wtri4)
                lb, ls = divmod(P["nch"] - 1, EXPB)
                if bi == lb:
                    nc.vector.tensor_mul(pt[:, ls], pt[:, ls], ctri4)

            def stage1_open(b, qi):
                c0 = max(0, qi - (MAXCH - 1))
                cs_list = list(range(c0, qi + 1))
                batches = [cs_list[i:i + EXPB] for i in range(0, len(cs_list),
                                                              EXPB)]
                edge_c = c0 if qi >= MAXCH - 1 else None
                mids = [c for c in cs_list if c != qi and c != edge_c]
                tail = ([edge_c] if edge_c is not None else []) + [qi]
                P = dict(b=b, qi=qi, c0=c0, nch=len(cs_list), pts=[],
                         batches=batches, edge_c=edge_c, mids=mids, tail=tail)
                for bi in range(min(2, len(batches))):
                    mm1_batch(P, bi)
                return P

            def stage1_rest(P):
                for bi in range(2, len(P["batches"])):
                    mm1_batch(P, bi)

            def pslice(P, c, g):
                bi, sj = divmod(c - P["c0"], EXPB)
                return P["pts"][bi][:, sj, g * 128:(g + 1) * 128]

            def stage3(P):
                # mm2, one contiguous accumulation group per head; masked
                # chunks last within each group
                po = [po_pool.tile([128, 2, D + 1], fp32, name=f"po{h}",
                                   tag=f"po{h}")
                      for h in range(2)]
                for g in range(G):
                    for idx, c in enumerate(P["mids"] + P["tail"]):
                        nc.tensor.matmul(
                            po[g // 2][:, g % 2],
                            pslice(P, c, g),
                            vsb[:, P["b"] * NKC + c],
                            start=(idx == 0),
                            stop=(idx == P["nch"] - 1),
                        )
                # normalize: reciprocal of the denominator columns, then a
                # per-head scalar multiply into the SBUF out tile
                osb = osb_pool.tile([128, GD], fp32, tag="osb")
                for h in range(2):
                    rec = rec_pool.tile([128, 2], fp32, name=f"rec{h}",
                                        tag=f"rec{h}")
                    nc.vector.reciprocal(rec, po[h][:, :, D:D + 1])
                    for gg in range(2):
                        g = h * 2 + gg
                        nc.vector.tensor_scalar_mul(
                            osb[:, g * 128:(g + 1) * 128],
                            po[h][:, gg, 0:D],
                            rec[:, gg:gg + 1],
                        )
                r0 = P["b"] * S + P["qi"] * 128
                nc.sync.dma_start(out_ext[r0:r0 + 128, :], osb)

            # pipeline: loads two quarters ahead, rope one quarter ahead of
            # the q blocks that consume them (rope op-groups interleaved
            # between this quarter's q blocks)
            steps = [(b, qq) for b in range(B) for qq in range(4)]
            allqi = [(b, qi) for b in range(B) for qi in range(NQB)]
            P = stage1_open(0, 0)
            stage1_rest(P)
            n = 0
            for i, (b, qq) in enumerate(steps):
                if i + 2 < len(steps):
                    loads(*steps[i + 2])
                if (b, qq) == (0, 2):
                    nc.sync.dma_start(
                        vsb[:, NKC:2 * NKC],
                        v_ext[:, NKC * (D + 1):].rearrange("p (c d) -> p c d",
                                                           d=D + 1))
                chunks = rope_chunks(*steps[i + 1]) if i + 1 < len(steps) else []
                for qi in range(qq * 4, qq * 4 + 4):
                    # next quarter's rope op-group must precede the stage1
                    # that consumes it at the quarter boundary
                    if chunks:
                        chunks.pop(0)()
                    Pn = (stage1_open(*allqi[n + 1])
                          if n + 1 < len(allqi) else None)
                    stage3(P)
                    if Pn is not None:
                        stage1_rest(Pn)
                    P = Pn
                    n += 1

    return nc


def _get_nc():
    global _CACHED_NC
    if _CACHED_NC is None:
        _CACHED_NC = _build_nc()
    return _CACHED_NC


def _host_tables(positions):
    """Rotary cos/sin caches in transposed-dup layout + triangular masks."""
    pos = positions.astype(np.float32)  # [NTOK]
    invf = (1.0 / (THETA ** (np.arange(HALF, dtype=np.float32) / HALF)))  # [64]
    ang = pos[None, :] * invf[:, None]  # [64, NTOK]
    c = np.cos(ang)
    s = np.sin(ang)
    cosd = np.concatenate([c, c], axis=0).astype(BF16)          # [128, NTOK]
    sind = np.concatenate([-s, s], axis=0).astype(BF16)         # [128, NTOK]
    # tile over the interleaved head dim -> [128, (b, qi, g, j)]
    cosd = np.ascontiguousarray(
        np.broadcast_to(cosd.reshape(128, B * NQB, 1, 128),
                        (128, B * NQB, G, 128)).reshape(128, NTOK * G))
    sind = np.ascontiguousarray(
        np.broadcast_to(sind.reshape(128, B * NQB, 1, 128),
                        (128, B * NQB, G, 128)).reshape(128, NTOK * G))
    p = np.arange(128)[:, None]
    f = np.arange(128)[None, :]
    ctri = (p <= f).astype(BF16)   # causal diagonal chunk: keep j<=i
    wtri = (f < p).astype(BF16)    # window edge chunk: keep i-j<WINDOW
    ctri4 = np.tile(ctri, (1, G))  # [128, 512]: per-head copies
    wtri4 = np.tile(wtri, (1, G))
    return cosd, sind, ctri4, wtri4


def _rot(xt):
    """Rotate-half along the (leading) d axis of a [d, token] array."""
    return np.concatenate([xt[HALF:], xt[:HALF]], axis=0)


def _run(inputs, trace=False):
    query = inputs["query"]
    key = inputs["key"]
    value = inputs["value"]
    positions = inputs["positions"]

    cosd, sind, ctri4, wtri4 = _host_tables(positions)
    qf = query.astype(BF16)
    kf = key.astype(BF16)
    vf = value.astype(BF16)
    ones = np.ones((NTOK, 1), dtype=BF16)

    def quarters(x):
        # [128, NTOK*G] -> [128, 8 quarters, 2048]
        return x.reshape(128, 8, 2048)

    in_maps = []
    for c in range(NCORES):
        # head-interleave to rows (b, qi, g, j), then transpose to [d, *]
        qc = qf[:, c * GD:(c + 1) * GD]
        q2 = (qc.reshape(B, NQB, 128, G, D)
              .transpose(0, 1, 3, 2, 4).reshape(NTOK * G, D))
        qt = np.ascontiguousarray(q2.T)
        kt = np.ascontiguousarray(kf[:, c * D:(c + 1) * D].T)
        qtr, ktr = _rot(qt), _rot(kt)
        # pack per quarter: qa = [q | cos4], qb = [qrot | sin4],
        # kc = [k | krot]
        qa_p = np.stack([quarters(qt), quarters(cosd)], axis=2)
        qb_p = np.stack([quarters(qtr), quarters(sind)], axis=2)
        kc_p = np.stack([kt.reshape(128, 8, 512), ktr.reshape(128, 8, 512)],
                        axis=2)
        in_maps.append({
            "qa": np.ascontiguousarray(qa_p.reshape(128, NTOK * G * 2)),
            "qb": np.ascontiguousarray(qb_p.reshape(128, NTOK * G * 2)),
            "kc": np.ascontiguousarray(kc_p.reshape(128, NTOK * 2)),
            "v": np.ascontiguousarray(
                np.concatenate([vf[:, c * D:(c + 1) * D], ones], axis=1)
                .reshape(NTOK // 128, 128, D + 1).transpose(1, 0, 2)
                .reshape(128, (NTOK // 128) * (D + 1))
            ),
            "ctri4": ctri4,
            "wtri4": wtri4,
        })

    nc = _get_nc()
    res = run_bass_kernel_spmd(nc, in_maps, core_ids=list(range(NCORES)),
                               trace=trace)
    out = np.concatenate([res.results[c]["out"] for c in range(NCORES)], axis=1)
    return out.astype(np.float32), res


def kernel(query, key, value, positions):
    out, _ = _run({"query": query, "key": key, "value": value,
                   "positions": positions},
                  trace=bool(os.environ.get("KERNEL_TRACE")))
    return out
